# revision 6
# baseline (speedup 1.0000x reference)
"""Trainium2 Bass kernel for the CPC loss (nn_CPC_292057776614).

Strategy (data-parallel over predictions, 8 cores, step-paired sharding):
  - Each core takes 1120 predictions as two step-pure segments: 448 preds
    of step X and 672 preds of step Y, with the (X, Y) pairing
      c0:(s0,s0) c1:(s0,s0) c2:(s0,s1) c3:(s1,s1)
      c4:(s1,s2) c5:(s2,s2) c6:(s4,s3) c7:(s4,s3)
    which exactly tiles all five steps' predictions across the cores and
    means every core DMAs only TWO 1280x1280 weight matrices (6.55 MB
    bf16) instead of five.
  - Host prep (index-derived packing + dtype/layout changes only):
    ctxT (gathered ctx rows, K-blocked transposed), encT (K-blocked
    transposed), the two weight matrices (K/M-blocked, transposed),
    per-row candidate column indices (uint16), a fixed 16-lane selection
    mask, and a valid-row mask.
  - Device, per core:
      stage A: predT[m,p] accumulated over 10 k-tiles in PSUM,
               bias added on ACT evacuation, bf16.
      stage B: dense scores pred @ enc^T in 7 PSUM chunks of 448 f32
               per 128-row tile; each chunk evacuated f32 to SBUF
               (alternating ACT/DVE).
      extraction: gpsimd indirect_copy pulls, for each 16-partition
               group, the group's 16x17 candidate columns ([128, 272]);
               a host-built one-hot 16-lane mask + DVE mul/reduce
               extracts each row's own 17 candidate scores -> [128, 17]
               per tile. All softmax math then runs on [128, 9*17]:
               maxneg over slots 1..16, M = max(d0, maxneg),
               sumexp via ACT Exp with accum_out, loss/corr, masked
               partition-reduce via ones-matmul.
  - Host sums the 8 per-core [loss_sum, correct_sum] pairs, / 8960.

Numerics: matmuls bf16 with f32 PSUM accumulation; scores extracted and
softmax'd in f32. No multiplicity handling needed: duplicate candidates
are extracted per-slot exactly like the reference's gather.
"""

import numpy as np
import ml_dtypes

import concourse.bass as bass
import concourse.mybir as mybir
import concourse.tile as tile
from concourse import bacc
from concourse.bass_utils import run_bass_kernel_spmd

BF16 = mybir.dt.bfloat16
F32 = mybir.dt.float32
I16 = mybir.dt.int16

# Problem constants (hardcoded; kernel.py must be self-contained).
B, G, D, S, NEG = 64, 7, 1280, 5, 16
CELLS = G * G            # 49
R = B * CELLS            # 3136 rows in ctx/enc
K17 = NEG + 1            # 17 candidates per prediction
STEP_LENS = [B * (G - 1 - s) * G for s in range(S)]     # [2688,2240,1792,1344,896]
P_TOTAL = sum(STEP_LENS)                                # 8960
N_CORES = 8
PC = P_TOTAL // N_CORES                                 # 1120 per core
NT = 9                                                  # p-tiles of 128
PP = NT * 128                                           # 1152 padded
KD = D // 128                                           # 10 k-tiles
ECH = 448                                               # score chunk (448*7=3136)
NE = R // ECH                                           # 7 chunks
KIDX = 32                                               # padded idx slots/tile (64B-aligned)
GW = 16 * KIDX                                          # 512 wrapped-gather width
SEGA, SEGB = 448, 672                                   # per-core segments
# (X, Y) step pair per core: 448 preds of X + 672 preds of Y
PAIRS = [(0, 0), (0, 0), (0, 1), (1, 1), (1, 2), (2, 2), (4, 3), (4, 3)]

_CACHE = {}

DEBUG = bool(int(__import__("os").environ.get("BASS_CPC_DEBUG", "0")))


def _build():
    """Build (and cache) the per-core Bass program. All 8 cores run the
    identical program on different data."""
    if "nc" in _CACHE:
        return _CACHE["nc"]

    nc = bacc.Bacc("TRN2", target_bir_lowering=False, debug=False)

    ctxT_d = nc.dram_tensor("ctxT", [128, KD, PP], BF16, kind="ExternalInput")
    encT_d = nc.dram_tensor("encT", [128, KD, R], BF16, kind="ExternalInput")
    W_d = nc.dram_tensor("Wb", [2 * KD, 128, KD, 128], BF16, kind="ExternalInput")
    bias_d = nc.dram_tensor("biasT", [128, 2 * KD], F32, kind="ExternalInput")
    idx_d = nc.dram_tensor("idxT", [128, NT * KIDX], I16, kind="ExternalInput")
    sel_d = nc.dram_tensor("selT", [128, GW], F32, kind="ExternalInput")
    vmask_d = nc.dram_tensor("vmask", [128, NT], F32, kind="ExternalInput")
    out_d = nc.dram_tensor("out", [1, 2], F32, kind="ExternalOutput")
    if DEBUG:
        ext_dbg = nc.dram_tensor("ext_dbg", [128, NT * KIDX], F32, kind="ExternalOutput")
        st_dbg = nc.dram_tensor("st_dbg", [128, 6 * NT], F32, kind="ExternalOutput")

    with tile.TileContext(nc) as tc:
        with (
            tc.tile_pool(name="const", bufs=1) as const,
            tc.tile_pool(name="spool", bufs=4) as spool,
            tc.tile_pool(name="scp", bufs=2) as scpool,
            tc.tile_pool(name="gp", bufs=2) as gpool,
            tc.tile_pool(name="psf", bufs=1, space="PSUM") as psf_pool,
        ):
            # ---- DMA priority order: stage-A-first-needed, then the rest ----
            ctxT_sb = const.tile([128, KD, PP], BF16)
            W_sb = const.tile([128, 2 * KD, KD, 128], BF16)
            encT_sb = const.tile([128, KD, R], BF16)
            for k in range(KD):
                nc.sync.dma_start(
                    out=ctxT_sb[:, k, 0:SEGA], in_=ctxT_d.ap()[:, k, 0:SEGA]
                )
            for m in range(KD):
                nc.sync.dma_start(out=W_sb[:, m, :, :], in_=W_d.ap()[m])
            for k in range(KD):
                nc.sync.dma_start(
                    out=ctxT_sb[:, k, SEGA:PP], in_=ctxT_d.ap()[:, k, SEGA:PP]
                )
            for m in range(KD, 2 * KD):
                nc.sync.dma_start(out=W_sb[:, m, :, :], in_=W_d.ap()[m])
            for n in range(NE):
                cols = slice(n * ECH, (n + 1) * ECH)
                for k in range(KD):
                    nc.sync.dma_start(
                        out=encT_sb[:, k, cols], in_=encT_d.ap()[:, k, cols]
                    )
            bias_sb = const.tile([128, 2 * KD], F32)
            nc.sync.dma_start(out=bias_sb[:], in_=bias_d.ap())
            idx_sb = const.tile([128, NT * KIDX], I16)
            nc.sync.dma_start(out=idx_sb[:], in_=idx_d.ap())
            sel_sb = const.tile([128, GW], F32)
            nc.sync.dma_start(out=sel_sb[:], in_=sel_d.ap())
            vmask_sb = const.tile([128, NT], F32)
            nc.sync.dma_start(out=vmask_sb[:], in_=vmask_d.ap())
            ones = const.tile([128, 1], F32)
            nc.vector.memset(ones[:], 1.0)

            predT_sb = const.tile([128, KD, PP], BF16)
            # zero the padded prediction columns so stage B stays finite
            nc.vector.memset(predT_sb[:, :, PC:PP], 0.0)

            ext_sb = const.tile([128, NT * KIDX], F32)
            junk_sb = const.tile([128, NT * K17], BF16)
            maxneg_sb = spool.tile([128, NT], F32)
            M_sb = spool.tile([128, NT], F32)
            negM_sb = spool.tile([128, NT], F32)
            sume_sb = spool.tile([128, NT], F32)

            # ---- PE pre-warm during the initial DMA wait (dead writes) ----
            with tc.tile_pool(name="warm", bufs=1, space="PSUM") as warm_pool:
                wps = warm_pool.tile([1, 1], F32)
                for _ in range(96):
                    nc.tensor.matmul(
                        wps[:], lhsT=ones[:], rhs=ones[:], start=True, stop=True
                    )

            # ---- stage A: predT = W^T-contract(ctxT) + bias ----
            with tc.tile_pool(name="psA", bufs=2, space="PSUM") as psA:
                for slot in range(2):
                    lo, chunks = (0, [(0, SEGA)]) if slot == 0 else (
                        SEGA, [(0, 336), (336, 336)]
                    )
                    for m in range(KD):
                        w = slot * KD + m
                        for (co, cw) in chunks:
                            pa = psA.tile([128, cw], F32, tag="pa")
                            for k in range(KD):
                                nc.tensor.matmul(
                                    pa[:],
                                    lhsT=W_sb[:, w, k, :],
                                    rhs=ctxT_sb[:, k, lo + co : lo + co + cw],
                                    start=(k == 0),
                                    stop=(k == KD - 1),
                                )
                            nc.scalar.activation(
                                predT_sb[:, m, lo + co : lo + co + cw],
                                pa[:],
                                mybir.ActivationFunctionType.Identity,
                                bias=bias_sb[:, w : w + 1],
                                scale=1.0,
                            )

            # ---- stage B + extraction + per-tile softmax stats ----
            with tc.tile_pool(name="psB", bufs=7, space="PSUM") as psB:
                for t in range(NT):
                    rows = slice(t * 128, (t + 1) * 128)
                    eb = t * KIDX
                    sc = scpool.tile([128, R], F32, tag="sc")
                    for n in range(NE):
                        cols = slice(n * ECH, (n + 1) * ECH)
                        pb = psB.tile([128, ECH], F32, tag="pb")
                        for k in range(KD):
                            nc.tensor.matmul(
                                pb[:],
                                lhsT=predT_sb[:, k, rows],
                                rhs=encT_sb[:, k, cols],
                                start=(k == 0),
                                stop=(k == KD - 1),
                            )
                        nc.vector.tensor_copy(sc[:, cols], pb[:])
                    # wrapped gather (gpsimd): row p gets, at 16k+(p%16),
                    # score[p, cand_k[group_base + p%16]]; mul+reduce also on
                    # gpsimd so the Vector queue stays pure-evac.
                    g = gpool.tile([128, GW], F32, tag="g")
                    nc.gpsimd.ap_gather(
                        out_ap=g[:],
                        in_ap=sc[:],
                        idxs_ap=idx_sb[:, eb : eb + KIDX],
                        channels=128,
                        num_elems=R,
                        d=1,
                        num_idxs=GW,
                    )
                    scr = gpool.tile([128, GW], F32, tag="scr")
                    nc.gpsimd.tensor_mul(scr[:], g[:], sel_sb[:])
                    nc.vector.reduce_sum(
                        ext_sb[:, eb : eb + KIDX].rearrange("p (k o) -> p k o", o=1),
                        scr[:].rearrange("p (k q) -> p k q", q=16),
                        axis=mybir.AxisListType.X,
                    )
                    # per-tile stats (Vector) + exp accumulation (Scalar)
                    nc.vector.reduce_max(
                        maxneg_sb[:, t : t + 1],
                        ext_sb[:, eb + 1 : eb + K17],
                        axis=mybir.AxisListType.X,
                    )
                    nc.vector.tensor_tensor(
                        out=M_sb[:, t : t + 1],
                        in0=ext_sb[:, eb : eb + 1],
                        in1=maxneg_sb[:, t : t + 1],
                        op=mybir.AluOpType.max,
                    )
                    nc.vector.tensor_scalar_mul(
                        negM_sb[:, t : t + 1], M_sb[:, t : t + 1], -1.0
                    )
                    nc.scalar.activation(
                        junk_sb[:, t * K17 : (t + 1) * K17],
                        ext_sb[:, eb : eb + K17],
                        mybir.ActivationFunctionType.Exp,
                        bias=negM_sb[:, t : t + 1],
                        scale=1.0,
                        accum_out=sume_sb[:, t : t + 1],
                    )

            # ---- final: loss/corr per prediction, masked, reduced ----
            ext3 = ext_sb[:].rearrange("p (t k) -> p t k", k=KIDX)
            d0v = ext3[:, :, 0:1]
            lnS_sb = spool.tile([128, NT], F32)
            nc.scalar.activation(
                lnS_sb[:], sume_sb[:], mybir.ActivationFunctionType.Ln
            )
            t1 = spool.tile([128, NT], F32)
            nc.vector.tensor_add(t1[:], lnS_sb[:], M_sb[:])
            lossp = spool.tile([128, NT], F32)
            nc.vector.tensor_sub(
                lossp[:].rearrange("p (t o) -> p t o", o=1),
                t1[:].rearrange("p (t o) -> p t o", o=1),
                d0v,
            )
            corrp = spool.tile([128, NT], F32)
            nc.vector.tensor_tensor(
                out=corrp[:].rearrange("p (t o) -> p t o", o=1),
                in0=d0v,
                in1=maxneg_sb[:].rearrange("p (t o) -> p t o", o=1),
                op=mybir.AluOpType.is_ge,
            )
            res_sb = const.tile([128, 2 * NT], F32)
            nc.vector.tensor_mul(res_sb[:, 0:NT], lossp[:], vmask_sb[:])
            nc.vector.tensor_mul(res_sb[:, NT : 2 * NT], corrp[:], vmask_sb[:])

            if DEBUG:
                nc.sync.dma_start(out=ext_dbg.ap(), in_=ext_sb[:])
                nc.sync.dma_start(out=st_dbg.ap()[:, 0:NT], in_=maxneg_sb[:])
                nc.sync.dma_start(out=st_dbg.ap()[:, 2 * NT : 3 * NT], in_=M_sb[:])
                nc.sync.dma_start(out=st_dbg.ap()[:, 3 * NT : 4 * NT], in_=sume_sb[:])
                nc.sync.dma_start(out=st_dbg.ap()[:, 4 * NT : 5 * NT], in_=lossp[:])
                nc.sync.dma_start(out=st_dbg.ap()[:, 5 * NT : 6 * NT], in_=corrp[:])

            # ---- final reduction ----
            fin = const.tile([128, 2], F32)
            nc.vector.reduce_sum(
                fin[:, 0:1], res_sb[:, 0:NT], axis=mybir.AxisListType.X
            )
            nc.vector.reduce_sum(
                fin[:, 1:2], res_sb[:, NT : 2 * NT], axis=mybir.AxisListType.X
            )
            pf = psf_pool.tile([1, 2], F32)
            nc.tensor.matmul(pf[:], lhsT=ones[:], rhs=fin[:], start=True, stop=True)
            out_sb = const.tile([1, 2], F32)
            nc.vector.tensor_copy(out_sb[:], pf[:])
            nc.sync.dma_start(out=out_d.ap(), in_=out_sb[:])

    nc.compile()
    _CACHE["nc"] = nc
    return nc


def _core_slices():
    """Per-core [(step, start, len), (step, start, len)] global slices."""
    offs = np.concatenate([[0], np.cumsum(STEP_LENS)]).astype(np.int64)
    ptr = {s: int(offs[s]) for s in range(S)}
    out = []
    for c in range(N_CORES):
        sx, sy = PAIRS[c]
        ax = ptr[sx]
        ptr[sx] += SEGA
        ay = ptr[sy]
        ptr[sy] += SEGB
        out.append(((sx, ax, SEGA), (sy, ay, SEGB)))
    for s in range(S):
        assert ptr[s] == int(offs[s + 1]), (s, ptr[s], offs[s + 1])
    return out


def _prep_in_maps(contexts, encodings, Wk_w, Wk_b, ctx_idx, cand_idx):
    ctx_flat = (
        np.asarray(contexts, dtype=np.float32)
        .reshape(R, D)
        .astype(ml_dtypes.bfloat16)
    )
    enc_flat = (
        np.asarray(encodings, dtype=np.float32)
        .reshape(R, D)
        .astype(ml_dtypes.bfloat16)
    )
    # encT blocked: [128, KD, R]
    encT = np.ascontiguousarray(enc_flat.reshape(R, KD, 128).transpose(2, 1, 0))
    # W blocked per step: [KD(m), 128(p), KD(k), 128(j)]
    Wt_all = np.asarray(Wk_w, dtype=np.float32).transpose(0, 2, 1)  # [S, d_in, d_out]
    Wt_all = Wt_all.astype(ml_dtypes.bfloat16)
    Wblk = {}
    for s in set(s for p in PAIRS for s in p):
        Wblk[s] = np.ascontiguousarray(
            Wt_all[s].reshape(KD, 128, KD, 128).transpose(2, 1, 0, 3)
        )
    Wb_np = np.asarray(Wk_b, dtype=np.float32)
    ctx_idx = np.asarray(ctx_idx, dtype=np.int64)
    cand_idx = np.asarray(cand_idx, dtype=np.int64)

    # fixed selection mask [128, GW]
    pmod = np.arange(128) % 16
    sel = (pmod[:, None] == (np.arange(GW) % 16)[None, :]).astype(np.float32)
    vmask = np.ascontiguousarray(
        (np.arange(PP) < PC).astype(np.float32).reshape(NT, 128).T
    )

    in_maps = []
    for (segx, segy) in _core_slices():
        ci = np.concatenate(
            [ctx_idx[a : a + ln] for (_, a, ln) in (segx, segy)]
        )  # [1120]
        ki = np.concatenate(
            [cand_idx[a : a + ln] for (_, a, ln) in (segx, segy)], axis=0
        )  # [1120, 17]
        ci_pad = np.zeros(PP, np.int64)
        ci_pad[:PC] = ci
        ctxT = np.ascontiguousarray(
            ctx_flat[ci_pad].reshape(PP, KD, 128).transpose(2, 1, 0)
        )  # [128, KD, PP]
        ki_pad = np.zeros((PP, KIDX), np.int16)
        ki_pad[:PC, :K17] = ki.astype(np.int16)
        idxT = np.ascontiguousarray(
            ki_pad.reshape(NT, 128, KIDX).transpose(1, 0, 2).reshape(128, NT * KIDX)
        )
        Wcat = np.ascontiguousarray(
            np.stack([Wblk[segx[0]], Wblk[segy[0]]]).reshape(
                2 * KD, 128, KD, 128
            )
        )
        biasT = np.ascontiguousarray(
            np.concatenate(
                [
                    Wb_np[segx[0]].reshape(KD, 128).T,
                    Wb_np[segy[0]].reshape(KD, 128).T,
                ],
                axis=1,
            ).astype(np.float32)
        )  # [128, 2*KD]
        in_maps.append(
            {
                "ctxT": ctxT,
                "encT": encT,
                "Wb": Wcat,
                "biasT": biasT,
                "idxT": idxT,
                "selT": sel,
                "vmask": vmask,
            }
        )
    return in_maps


def _install_ntff_hook():
    """Provide antenv.axon_hooks if the image lacks it, so trace=True can
    capture NTFF profiles through the injected libaxon_pjrt.so."""
    import sys
    import types
    import ctypes
    import contextlib
    import os

    try:
        from antenv.axon_hooks import get_axon_ntff_profile_hook  # noqa: F401

        return
    except ImportError:
        pass
    so_path = "/opt/axon/libaxon_pjrt.so"
    if not os.path.exists(so_path):
        return
    lib = ctypes.CDLL(so_path)
    if not hasattr(lib, "axon_start_nrt_profile"):
        return
    lib.axon_start_nrt_profile.argtypes = [
        ctypes.POINTER(ctypes.c_int64),
        ctypes.c_size_t,
    ]
    lib.axon_start_nrt_profile.restype = ctypes.c_int64
    lib.axon_stop_nrt_profile.argtypes = [ctypes.c_char_p]
    lib.axon_stop_nrt_profile.restype = ctypes.c_int64

    @contextlib.contextmanager
    def _hook(output_dir, device_ids):
        import jax

        jax.devices()
        if device_ids:
            ids = (ctypes.c_int64 * len(device_ids))(*device_ids)
            rc = lib.axon_start_nrt_profile(ids, len(device_ids))
        else:
            rc = lib.axon_start_nrt_profile(None, 0)
        if rc != 0:
            raise RuntimeError(f"axon_start_nrt_profile rc={rc}")
        try:
            yield
        finally:
            n = lib.axon_stop_nrt_profile(str(output_dir).encode())
            print(f"ntff profile: {n} file(s) written to {output_dir}")

    mod = types.ModuleType("antenv.axon_hooks")
    mod.get_axon_ntff_profile_hook = lambda: _hook
    mod.set_axon_ntff_profile_hook = lambda h: None
    sys.modules["antenv.axon_hooks"] = mod


def run(inputs, trace=False, **kwargs):
    """Run the SPMD kernel; returns (loss, correct, BassKernelResults)."""
    if trace:
        _install_ntff_hook()
    nc = _build()
    in_maps = _prep_in_maps(**inputs)
    res = run_bass_kernel_spmd(
        nc, in_maps, core_ids=list(range(N_CORES)), trace=trace, **kwargs
    )
    sums = np.stack([r["out"].reshape(2) for r in res.results])  # [8, 2]
    tot = sums.sum(axis=0, dtype=np.float64)
    loss = np.float32(tot[0] / P_TOTAL)
    correct = np.float32(tot[1] / P_TOTAL)
    return loss, correct, res


def kernel(**inputs):
    loss, correct, _ = run(inputs, trace=False)
    return loss, correct
